# revision 40
# baseline (speedup 1.0000x reference)
"""Multi-head attention (B=4, S=2048, D=1024, H=16) on 8 TRN2 NeuronCores.

Sharding: DP=4 over batch x TP=2 over heads. Core c handles batch c//2 and
heads 8*(c%2) .. 8*(c%2)+8. Each core computes a partial output [S, D] (its
heads' contribution to the out-projection); the host sums the two TP partials
per batch and adds the output bias.

Key compaction: the key-padding mask removes ~half the keys, so the host
gathers unmasked key rows per batch (padded to a multiple of 128). k/v
projections and attention only touch NKV ~= S/2 keys; padding keys carry a
-1e9 additive bias fused into the exp so they contribute exactly 0.

Schedule: one continuous software pipeline over all (qc, hp, kc) steps --
scores pair -> exp (ACT) -> ctx pair two steps later -- with the q/k/v/out
projections injected between steps as 8-matmul "filler" groups, so the PE
never drains at an hp boundary and the exp pipeline starts as soon as the
first q/k chunks are projected. DMA loads are ordered and laid out
(qc-major xq, key-major xkv, head-major wq/wk) so the first matmul only
waits for ~1.3 MB. PSUM->SBUF copies for the out-projection run on the idle
GpSimd/Pool engine; softmax normalization (copy+recip+mul) stays on DVE.

On-chip layouts (matmul operands bf16, accumulation fp32 in PSUM):
  qT/kT : [hd, seq] with the two heads of a pair stacked on partitions
          (0-63 / 64-127) -> the scores matmuls (K=64) auto-pack into PE
          row-groups and run concurrently.
  scoresT[keys, q]: exp runs on ScalarE with fused scale + per-key mask bias,
          one op per [128, 1024] 2-bank PSUM tile covering both heads.
  v_aug : [keys, v | ones(64)] -> the ctx matmul accumulates ctxT (rows 0-63)
          and the softmax denominator replicated across rows 64-127, so the
          normalization is a shift-free fast-reciprocal + multiply on VectorE.
"""

import sys

sys.path.insert(0, "/opt/trn_rl_repo")

import numpy as np
import ml_dtypes

B, S, D, H = 4, 2048, 1024, 16
HD = D // H
SCALE = 1.0 / float(np.sqrt(HD))
NEG = -1e9

DP = 4  # batch shards
TP = 2  # head-group shards
HL = H // TP  # heads per core (8)
DL = HL * HD  # local head dims per core (512)
N_HP = HL // 2  # head pairs per core (4)
QCH = 512  # q chunk (free dim of score matmuls)
KV_P = 128  # key chunk (partition dim of scoresT)
KC8 = D // 128  # contraction chunks for projections (8)
NQC = S // QCH  # q chunks (4)
CTX_LAG = 5  # ctx pairs trail scores pairs by this many steps

bf16 = ml_dtypes.bfloat16


def _build(nkv, with_bias=True):
    from concourse import bacc
    import concourse.mybir as mybir
    from concourse.tile import TileContext

    dt = mybir.dt
    f32 = dt.float32
    b16 = dt.bfloat16
    EXP = mybir.ActivationFunctionType.Exp

    nkc = nkv // KV_P  # key chunks (9 for nkv=1152)
    # kproj blocks of <=512 keys
    kblocks = []
    off = 0
    while off < nkv:
        n = min(512, nkv - off)
        kblocks.append((off, n))
        off += n
    nkb = len(kblocks)

    nc = bacc.Bacc(trn_type="TRN2")

    # packed DRAM layouts (match SBUF exactly; every dma_start contiguous
    # per partition)
    xq_d = nc.dram_tensor("xq", (128, NQC * KC8 * QCH), b16, kind="ExternalInput").ap()
    xkv_d = nc.dram_tensor("xkv", (128, nkc * KC8 * KV_P), b16, kind="ExternalInput").ap()
    wq_d = nc.dram_tensor("wqt", (128, N_HP * KC8 * 128), b16, kind="ExternalInput").ap()
    wk_d = nc.dram_tensor("wkt", (128, N_HP * KC8 * 128), b16, kind="ExternalInput").ap()
    wv_d = nc.dram_tensor("wvt", (128, KC8 * DL), b16, kind="ExternalInput").ap()
    wo_d = nc.dram_tensor("wot", (128, (DL // 128) * D), b16, kind="ExternalInput").ap()
    mb_d = nc.dram_tensor("mbias", (nkv,), f32, kind="ExternalInput").ap()
    if with_bias:
        bq_d = nc.dram_tensor("bq", (1, DL), b16, kind="ExternalInput").ap()
        bk_d = nc.dram_tensor("bk", (1, DL), b16, kind="ExternalInput").ap()
        bv_d = nc.dram_tensor("bv", (1, DL), b16, kind="ExternalInput").ap()
    out_d = nc.dram_tensor("out", (S, D), f32, kind="ExternalOutput").ap()

    with TileContext(nc) as tc:
        with (
            tc.tile_pool(name="persist", bufs=1) as pp,
            tc.tile_pool(name="psum", bufs=2, space="PSUM") as ps2,
            tc.tile_pool(name="etile", bufs=8) as ep,
            tc.tile_pool(name="work", bufs=6) as wp,
            tc.tile_pool(name="ob", bufs=3) as obp,
        ):
            # ---- persistent SBUF tensors ----
            xq_sb = pp.tile([128, NQC, KC8, QCH], b16, tag="xq")
            xkv_sb = pp.tile([128, nkc, KC8, KV_P], b16, tag="xkv")
            wq_sb = pp.tile([128, N_HP, KC8, 128], b16, tag="wq")
            wk_sb = pp.tile([128, N_HP, KC8, 128], b16, tag="wk")
            wv_sb = pp.tile([128, KC8, DL], b16, tag="wv")
            wo_sb = pp.tile([128, DL // 128, D], b16, tag="wo")
            mb_sb = pp.tile([128, nkc], f32, tag="mb")
            qt_sb = pp.tile([128, N_HP, S], b16, tag="qt")
            kt_sb = pp.tile([128, N_HP, nkv], b16, tag="kt")
            # v_aug: [key_part, key_chunk, head, 64 v | 64 ones]
            v_sb = pp.tile([128, nkc, HL, 128], b16, tag="v")
            ctx_sb = pp.tile([128, N_HP, S], b16, tag="ctx")
            if with_bias:
                xq1_sb = pp.tile([1, S], b16, tag="xq1")
                xkv1_sb = pp.tile([1, nkv], b16, tag="xkv1")
                wq1_sb = pp.tile([1, DL], b16, tag="wq1")
                wk1_sb = pp.tile([1, DL], b16, tag="wk1")
                wv1_sb = pp.tile([1, DL], b16, tag="wv1")

            # ---- DMA loads, priority order: everything the first few
            # compute groups need goes first; the rest streams behind ----
            # Few, fat dma_starts: the SP sequencer issues each dma_start
            # serially (~0.6us apiece), so the issue count gates how early
            # the later tensors even begin transferring. First-needed
            # tensors are split in halves so the first projection matmuls
            # can start on the first piece.
            nc.sync.dma_start(mb_sb[:], mb_d.rearrange("(kc p) -> p kc", p=128))
            for h0, h1 in ((0, 4), (4, 8)):
                nc.sync.dma_start(
                    wq_sb[:, 0, h0:h1],
                    wq_d[:, h0 * 128 : h1 * 128].rearrange("p (kc c) -> p kc c", c=128),
                )
                nc.sync.dma_start(
                    xq_sb[:, 0, h0:h1],
                    xq_d[:, h0 * QCH : h1 * QCH].rearrange("p (kc c) -> p kc c", c=QCH),
                )
            nc.sync.dma_start(
                wk_sb[:, 0], wk_d[:, 0 : KC8 * 128].rearrange("p (kc c) -> p kc c", c=128)
            )
            nc.sync.dma_start(
                xkv_sb[:, 0:4],
                xkv_d[:, 0 : 4 * KC8 * KV_P].rearrange(
                    "p (mt kc c) -> p mt kc c", kc=KC8, c=KV_P
                ),
            )
            nc.sync.dma_start(
                wv_sb[:], wv_d.rearrange("p (kc c) -> p kc c", c=DL)
            )
            if with_bias:
                nc.sync.dma_start(wq1_sb[:], bq_d)
                nc.sync.dma_start(wk1_sb[:], bk_d)
                nc.sync.dma_start(wv1_sb[:], bv_d)
            nc.sync.dma_start(
                xkv_sb[:, 4:nkc],
                xkv_d[:, 4 * KC8 * KV_P :].rearrange(
                    "p (mt kc c) -> p mt kc c", kc=KC8, c=KV_P
                ),
            )
            nc.sync.dma_start(
                wk_sb[:, 1:N_HP],
                wk_d[:, KC8 * 128 :].rearrange("p (hp kc c) -> p hp kc c", kc=KC8, c=128),
            )
            nc.sync.dma_start(
                wq_sb[:, 1:N_HP],
                wq_d[:, KC8 * 128 :].rearrange("p (hp kc c) -> p hp kc c", kc=KC8, c=128),
            )
            nc.sync.dma_start(
                xq_sb[:, 1:NQC],
                xq_d[:, KC8 * QCH :].rearrange(
                    "p (qc kc c) -> p qc kc c", kc=KC8, c=QCH
                ),
            )
            nc.sync.dma_start(
                wo_sb[:],
                wo_d.rearrange("p (k c) -> p k c", c=D),
            )

            # constants: ones blocks of v_aug (Pool engine; per-chunk so the
            # first ctx isn't gated on one huge memset). Even heads are
            # [v | ones] (ctx rows 0:63, den 64:127); odd heads are flipped
            # [ones | v] so every DVE op in the normalization reads
            # partition-aligned operands.
            for mt in range(nkc):
                nc.gpsimd.memset(v_sb[:, mt, 0:HL:2, 64:128], 1.0)
                nc.gpsimd.memset(v_sb[:, mt, 1:HL:2, 0:64], 1.0)
            if with_bias:
                nc.vector.memset(xq1_sb[:], 1.0)
                nc.vector.memset(xkv1_sb[:], 1.0)

            # warm the ACT exp table during the DMA lead-in (first use of an
            # activation function costs a ~1.3us table load)
            warm = wp.tile([128, 1], b16, tag="warm", name="warm")
            nc.scalar.activation(warm[:], mb_sb[:, 0:1], EXP)



            # ---- group emitters (each: ~8 matmuls + one copy) ----
            # PSUM budget (8 banks): tag "s" 2 bufs x 2 banks shared by the
            # short-lived tiles (scores, proj, outproj halves); tag "c"
            # 2 bufs x 2 banks for the long-lived ctx accumulators.
            def qproj(hp, qc):
                """qt[:, hp, qc*512:+512] = wq_hp.T @ xq_qc."""
                ps = ps2.tile([128, 512], f32, tag="s", name="ps_q")
                for kc in range(KC8):
                    nc.tensor.matmul(
                        ps[:],
                        lhsT=wq_sb[:, hp, kc, :],
                        rhs=xq_sb[:, qc, kc, :],
                        start=(kc == 0),
                        stop=(not with_bias and kc == KC8 - 1),
                    )
                if with_bias:
                    nc.tensor.matmul(
                        ps[:],
                        lhsT=wq1_sb[:, hp * 128 : hp * 128 + 128],
                        rhs=xq1_sb[:, qc * QCH : qc * QCH + QCH],
                        start=False,
                        stop=True,
                    )
                nc.vector.tensor_copy(
                    out=qt_sb[:, hp, qc * QCH : qc * QCH + QCH], in_=ps[:]
                )

            def kproj(hp, kb):
                """kt[:, hp, off:off+n] for key block kb."""
                off, n = kblocks[kb]
                m0 = off // KV_P
                nmt = n // KV_P
                ps = ps2.tile([128, 512], f32, tag="s", name="ps_k")
                for kc in range(KC8):
                    nc.tensor.matmul(
                        ps[:, :n],
                        lhsT=wk_sb[:, hp, kc, :],
                        rhs=xkv_sb[:, m0 : m0 + nmt, kc, :],
                        start=(kc == 0),
                        stop=(not with_bias and kc == KC8 - 1),
                    )
                if with_bias:
                    nc.tensor.matmul(
                        ps[:, :n],
                        lhsT=wk1_sb[:, hp * 128 : hp * 128 + 128],
                        rhs=xkv1_sb[:, off : off + n],
                        start=False,
                        stop=True,
                    )
                nc.vector.tensor_copy(out=kt_sb[:, hp, off : off + n], in_=ps[:, :n])

            def vproj(mt):
                """v[keys mt*128:+128, all 8 heads] into v_sb."""
                ps = ps2.tile([128, 512], f32, tag="s", name="ps_v")
                for kc in range(KC8):
                    nc.tensor.matmul(
                        ps[:],
                        lhsT=xkv_sb[:, mt, kc, :],
                        rhs=wv_sb[:, kc, :],
                        start=(kc == 0),
                        stop=(not with_bias and kc == KC8 - 1),
                    )
                if with_bias:
                    nc.tensor.matmul(
                        ps[:],
                        lhsT=xkv1_sb[:, mt * KV_P : mt * KV_P + KV_P],
                        rhs=wv1_sb[:],
                        start=False,
                        stop=True,
                    )
                psv = ps[:].rearrange("p (h e) -> p h e", h=HL)
                nc.vector.tensor_copy(out=v_sb[:, mt, 0:HL:2, 0:64], in_=psv[:, 0:HL:2])
                nc.vector.tensor_copy(out=v_sb[:, mt, 1:HL:2, 64:128], in_=psv[:, 1:HL:2])

            def outproj(rt):
                """Full out-projection for row-tile rt (128 q rows): matmuls
                on PE into a "c"-tag tile (shares the rotation with the ctx
                accumulators without perturbing the scores slots), one copy
                on DVE, store via sync DMA."""
                rs = slice(rt * 128, rt * 128 + 128)
                ps = ps2.tile([128, D], f32, tag="c", name="ps_o")
                for nj in range(D // 512):
                    ns = slice(nj * 512, nj * 512 + 512)
                    for khp in range(N_HP):
                        nc.tensor.matmul(
                            ps[:, ns],
                            lhsT=ctx_sb[:, khp, rs],
                            rhs=wo_sb[:, khp, ns],
                            start=(khp == 0),
                            stop=(khp == N_HP - 1),
                        )
                ob = obp.tile([128, D], f32, tag="ob", name="ob")
                nc.vector.tensor_copy(out=ob[:], in_=ps[:])
                nc.sync.dma_start(out_d[rs, :], ob[:])

            # ---- main attention pipeline ----
            steps = [(qc, hp, kc) for qc in range(NQC) for hp in range(N_HP) for kc in range(nkc)]
            nsteps = len(steps)

            # filler schedule: {step_index: [callables]} -- each injected
            # between attention steps, at most ~1 group per step.
            fill = {t: [] for t in range(nsteps + 1)}

            def at(qc, hp, kc):
                return (qc * N_HP + hp) * nkc + min(kc, nkc - 1)

            # Placement rules (consumer must come later in PE program order):
            #   kproj(hp, kb) before scores(qc0, hp, kc=4*kb)
            #   qproj(hp, qc) before scores(qc, hp, kc=0)
            #   vproj(mt)     before ctx(0, 0, mt) at step mt+CTX_LAG
            #   outproj(qc,·) after norm(qc, hp3), i.e. in the (qc+1, ·) iters
            # Pre-phase covers qproj(0,0), kproj(0,0), vproj(0..1).
            fill[at(0, 0, 0)] += [lambda: vproj(0), lambda: vproj(1)]
            for kb in range(1, nkb):  # rest of hp0's keys
                fill[at(0, 0, max(0, 4 * kb - 3))] += [lambda k=kb: kproj(0, k)]
            for mt in range(2, nkc):  # v for all key chunks
                fill[at(0, 0, mt)] += [lambda m=mt: vproj(m)]
            fill[at(0, 0, 5)] += [lambda: qproj(1, 0)]
            fill[at(0, 0, 7)] += [lambda: kproj(1, 0)]
            for hp in range(1, N_HP):
                for kb in range(1, nkb):
                    fill[at(0, hp, max(0, 4 * kb - 3))] += [
                        lambda h=hp, k=kb: kproj(h, k)
                    ]
                if hp + 1 < N_HP:
                    fill[at(0, hp, 4)] += [lambda h=hp + 1: qproj(h, 0)]
                    fill[at(0, hp, 6)] += [lambda h=hp + 1: kproj(h, 0)]
            # remaining qproj groups: one per iteration, one iteration ahead
            # of consumption
            for qc in range(1, NQC):
                for hp in range(N_HP):
                    lin = qc * N_HP + hp - 1
                    pq, ph = divmod(lin, N_HP)
                    fill[at(pq, ph, 3 if pq > 0 else 5)] += [
                        lambda h=hp, q=qc: qproj(h, q)
                    ]
            # out-projection row-tiles spread over the next qc's iterations;
            # kept >= one iteration past norm(qc, hp3) so the PE never waits
            # on the DVE normalization chain. rt11 is held back for the tail:
            # it depends only on qc2's norms, so it overlaps the final
            # normalization chain that everything after it must wait for.
            for qc in range(NQC - 1):
                rts = [qc * 4 + i for i in range(4)]
                fill[at(qc + 1, 1, 1)] += [lambda r=rts[0]: outproj(r)]
                fill[at(qc + 1, 1, 6)] += [lambda r=rts[1]: outproj(r)]
                fill[at(qc + 1, 2, 1)] += [lambda r=rts[2]: outproj(r)]
                if qc != NQC - 2:
                    fill[at(qc + 1, 2, 6)] += [lambda r=rts[3]: outproj(r)]

            # pre-phase: minimum work to start the first scores/exp. The
            # junk matmuls between the groups keep the Tensor engine busy
            # through the DMA waits (it runs at ~half clock until it has
            # executed continuously for ~3us, and idle gaps reset the ramp).
            jnk = wp.tile([128, 512], b16, tag="jnk", name="jnk")
            nc.vector.memset(jnk[:], 0.0)

            def junk_mms(n):
                jps = ps2.tile([128, 512], f32, tag="s", name="jps")
                for i in range(n):
                    nc.tensor.matmul(
                        jps[:],
                        lhsT=jnk[:, 0:128],
                        rhs=jnk[:],
                        start=(i == 0),
                        stop=(i == n - 1),
                    )

            qproj(0, 0)
            junk_mms(8)
            kproj(0, 0)
            junk_mms(6)

            cc_tiles = {}  # (qc, hp) -> psum tile
            pending = []  # (e01, qc, hp, kc)

            def ctx_mm(e01, qc, hp, kc):
                key = (qc, hp)
                if key not in cc_tiles:
                    cc_tiles[key] = ps2.tile([128, 2 * QCH], f32, tag="c", name="cc")
                cc = cc_tiles[key]
                nc.tensor.matmul(
                    cc[:, 0:QCH],
                    lhsT=v_sb[:, kc, 2 * hp, :],
                    rhs=e01[:, 0:QCH],
                    start=(kc == 0),
                    stop=(kc == nkc - 1),
                )
                nc.tensor.matmul(
                    cc[:, QCH : 2 * QCH],
                    lhsT=v_sb[:, kc, 2 * hp + 1, :],
                    rhs=e01[:, QCH : 2 * QCH],
                    start=(kc == 0),
                    stop=(kc == nkc - 1),
                )
                if kc == nkc - 1:
                    norm(qc, hp)
                    del cc_tiles[key]

            def norm(qc, hp):
                """ctx_sb[:, hp, qs] = ctx / den.
                Head-even (c0): ctx rows 0:63, den rows 64:127; head-odd (c1)
                flipped: den 0:63, ctx 64:127. Both dens relocate into one
                [128, 512] SBUF tile -> a single full-width
                reciprocal_approx_fast (base-partition-0, unshifted) covers
                both heads, and both multiplies read partition-aligned
                operands."""
                qs = slice(qc * QCH, qc * QCH + QCH)
                cc = cc_tiles[(qc, hp)]
                c0 = cc[:, 0:QCH]
                c1 = cc[:, QCH : 2 * QCH]
                den = wp.tile([128, QCH], f32, tag="den", name="den")
                nc.vector.tensor_copy(out=den[0:64, :], in_=c0[64:128, :])
                nc.vector.tensor_copy(out=den[64:128, :], in_=c1[0:64, :])
                rc = wp.tile([128, QCH], f32, tag="rc", name="rc")
                nc.vector.reciprocal_approx_fast(rc[:], den[:])
                nc.vector.tensor_mul(
                    out=ctx_sb[0:64, hp, qs], in0=c0[0:64, :], in1=rc[0:64, :]
                )
                nc.vector.tensor_mul(
                    out=ctx_sb[64:128, hp, qs], in0=c1[64:128, :], in1=rc[64:128, :]
                )

            def scores_exp(t):
                qc, hp, kc = steps[t]
                qs = slice(qc * QCH, qc * QCH + QCH)
                ks = slice(kc * KV_P, kc * KV_P + KV_P)
                s01 = ps2.tile([128, 2 * QCH], f32, tag="s", name="s01")
                nc.tensor.matmul(
                    s01[:, 0:QCH],
                    lhsT=kt_sb[0:64, hp, ks],
                    rhs=qt_sb[0:64, hp, qs],
                )
                nc.tensor.matmul(
                    s01[:, QCH : 2 * QCH],
                    lhsT=kt_sb[64:128, hp, ks],
                    rhs=qt_sb[64:128, hp, qs],
                )
                e01 = ep.tile([128, 2 * QCH], b16, tag="e", name="e01")
                nc.scalar.activation(
                    e01[:],
                    s01[:],
                    EXP,
                    bias=mb_sb[:, kc : kc + 1],
                    scale=SCALE,
                )
                pending.append((e01, qc, hp, kc))

            # steps run in PAIRS: both scores+exp first (filling both PSUM
            # scores slots, so ACT has ~2us of queued exp work), then the
            # ctx pops, then fills. This keeps ACT fed across a fill group
            # and keeps the PE from waiting on exp at the next pair.
            # Pops stay before fills: a fill may read state (ctx_sb via
            # norm) that the popped ctx/norm writes.
            for p in range(0, nsteps, 2):
                pair = [t for t in (p, p + 1) if t < nsteps]
                for t in pair:
                    scores_exp(t)
                while len(pending) > CTX_LAG:
                    ctx_mm(*pending.pop(0))
                for t in pair:
                    for f in fill.get(t, ()):
                        f()

            for p in pending:
                ctx_mm(*p)

            # tail: rt11 first (ready immediately — fills the PE while the
            # DVE runs the final norm), then the last q-chunk's rows
            outproj(4 * (NQC - 1) - 1)
            for rt in range(4 * (NQC - 1), 4 * NQC):
                outproj(rt)

    nc.finalize()
    return nc


def _host_prep(x, mask, wq, bq, wk, bk, wv, bv, wo):
    x = np.asarray(x, dtype=np.float32)
    mask = np.asarray(mask)
    # per-batch gather of unmasked keys
    idxs = [np.nonzero(mask[b])[0] for b in range(B)]
    nmax = max(1, max(len(i) for i in idxs))
    nkv = min(S, ((nmax + KV_P - 1) // KV_P) * KV_P)
    nkc = nkv // KV_P
    with_bias = bool(
        np.any(np.asarray(bq)) or np.any(np.asarray(bk)) or np.any(np.asarray(bv))
    )

    def pack_x(xt, inner):
        """[D, n] -> [128, (n//inner) * KC8 * inner]: outer-major blocks of
        `inner` columns, kc-major within a block, contiguous per partition."""
        d, n = xt.shape
        nb = n // inner
        a = xt.reshape(KC8, 128, nb, inner).transpose(1, 2, 0, 3)
        return np.ascontiguousarray(a).reshape(128, nb * KC8 * inner).astype(bf16)

    def pack_w(wt):
        """[D, 512] -> [128, N_HP * KC8 * 128] (hp-major, kc, 128)."""
        a = wt.reshape(KC8, 128, N_HP, 128).transpose(1, 2, 0, 3)
        return np.ascontiguousarray(a).reshape(128, N_HP * KC8 * 128).astype(bf16)

    def pack_kc(a, kcount):
        """[kcount*128, n] -> [128, kcount*n] kc-major per partition."""
        k128, n = a.shape
        return (
            np.ascontiguousarray(a.reshape(kcount, 128, n).transpose(1, 0, 2))
            .reshape(128, kcount * n)
            .astype(bf16)
        )

    in_maps = []
    for c in range(DP * TP):
        b, g = c // TP, c % TP
        sl = slice(g * DL, g * DL + DL)

        idx = idxs[b]
        xg = np.zeros((nkv, D), dtype=np.float32)
        xg[: len(idx)] = x[b][idx]

        mbias = np.full((nkv,), NEG, dtype=np.float32)
        mbias[: len(idx)] = 0.0

        im = {
            "xq": pack_x(x[b].T, QCH),
            "xkv": pack_x(xg.T, KV_P),
            "wqt": pack_w(np.asarray(wq)[sl, :].T),
            "wkt": pack_w(np.asarray(wk)[sl, :].T),
            "wvt": pack_kc(np.asarray(wv)[sl, :].T, KC8),
            "wot": pack_kc(np.asarray(wo)[:, sl].T, DL // 128),
            "mbias": mbias,
        }
        if with_bias:
            im["bq"] = np.asarray(bq)[None, sl].astype(bf16)
            im["bk"] = np.asarray(bk)[None, sl].astype(bf16)
            im["bv"] = np.asarray(bv)[None, sl].astype(bf16)
        in_maps.append(im)
    return nkv, with_bias, in_maps


def kernel(x, mask, wq, bq, wk, bk, wv, bv, wo, bo):
    from concourse.bass_utils import run_bass_kernel_spmd

    nkv, with_bias, in_maps = _host_prep(x, mask, wq, bq, wk, bk, wv, bv, wo)
    nc = _build(nkv, with_bias)
    res = run_bass_kernel_spmd(nc, in_maps, core_ids=list(range(DP * TP)))

    out = np.empty((B, S, D), dtype=np.float32)
    bo = np.asarray(bo, dtype=np.float32)
    for b in range(B):
        out[b] = res.results[b * TP]["out"] + res.results[b * TP + 1]["out"] + bo
    return out


# revision 42
# speedup vs baseline: 1.0199x; 1.0199x over previous
"""Multi-head attention (B=4, S=2048, D=1024, H=16) on 8 TRN2 NeuronCores.

Sharding: DP=4 over batch x TP=2 over heads. Core c handles batch c//2 and
heads 8*(c%2) .. 8*(c%2)+8. Each core computes a partial output [S, D] (its
heads' contribution to the out-projection); the host sums the two TP partials
per batch and adds the output bias.

Key compaction: the key-padding mask removes ~half the keys, so the host
gathers unmasked key rows per batch (padded to a multiple of 128). k/v
projections and attention only touch NKV ~= S/2 keys; padding keys carry a
-1e9 additive bias fused into the exp so they contribute exactly 0.

Schedule: one continuous software pipeline over all (qc, hp, kc) steps --
scores pair -> exp (ACT) -> ctx pair two steps later -- with the q/k/v/out
projections injected between steps as 8-matmul "filler" groups, so the PE
never drains at an hp boundary and the exp pipeline starts as soon as the
first q/k chunks are projected. DMA loads are ordered and laid out
(qc-major xq, key-major xkv, head-major wq/wk) so the first matmul only
waits for ~1.3 MB. PSUM->SBUF copies for the out-projection run on the idle
GpSimd/Pool engine; softmax normalization (copy+recip+mul) stays on DVE.

On-chip layouts (matmul operands bf16, accumulation fp32 in PSUM):
  qT/kT : [hd, seq] with the two heads of a pair stacked on partitions
          (0-63 / 64-127) -> the scores matmuls (K=64) auto-pack into PE
          row-groups and run concurrently.
  scoresT[keys, q]: exp runs on ScalarE with fused scale + per-key mask bias,
          one op per [128, 1024] 2-bank PSUM tile covering both heads.
  v_aug : [keys, v | ones(64)] -> the ctx matmul accumulates ctxT (rows 0-63)
          and the softmax denominator replicated across rows 64-127, so the
          normalization is a shift-free fast-reciprocal + multiply on VectorE.
"""

import sys

sys.path.insert(0, "/opt/trn_rl_repo")

import numpy as np
import ml_dtypes

B, S, D, H = 4, 2048, 1024, 16
HD = D // H
SCALE = 1.0 / float(np.sqrt(HD))
NEG = -1e9

DP = 4  # batch shards
TP = 2  # head-group shards
HL = H // TP  # heads per core (8)
DL = HL * HD  # local head dims per core (512)
N_HP = HL // 2  # head pairs per core (4)
QCH = 512  # q chunk (free dim of score matmuls)
KV_P = 128  # key chunk (partition dim of scoresT)
KC8 = D // 128  # contraction chunks for projections (8)
NQC = S // QCH  # q chunks (4)
CTX_LAG = 5  # ctx pairs trail scores pairs by this many steps

bf16 = ml_dtypes.bfloat16


def _build(nkv, with_bias=True):
    from concourse import bacc
    import concourse.mybir as mybir
    from concourse.tile import TileContext

    dt = mybir.dt
    f32 = dt.float32
    b16 = dt.bfloat16
    EXP = mybir.ActivationFunctionType.Exp

    nkc = nkv // KV_P  # key chunks (9 for nkv=1152)
    # kproj blocks of <=512 keys
    kblocks = []
    off = 0
    while off < nkv:
        n = min(512, nkv - off)
        kblocks.append((off, n))
        off += n
    nkb = len(kblocks)

    nc = bacc.Bacc(trn_type="TRN2")

    # packed DRAM layouts (match SBUF exactly; every dma_start contiguous
    # per partition)
    xq_d = nc.dram_tensor("xq", (128, NQC * KC8 * QCH), b16, kind="ExternalInput").ap()
    xkv_d = nc.dram_tensor("xkv", (128, nkc * KC8 * KV_P), b16, kind="ExternalInput").ap()
    wq_d = nc.dram_tensor("wqt", (128, N_HP * KC8 * 128), b16, kind="ExternalInput").ap()
    wk_d = nc.dram_tensor("wkt", (128, N_HP * KC8 * 128), b16, kind="ExternalInput").ap()
    wv_d = nc.dram_tensor("wvt", (128, KC8 * DL), b16, kind="ExternalInput").ap()
    wo_d = nc.dram_tensor("wot", (128, (DL // 128) * D), b16, kind="ExternalInput").ap()
    mb_d = nc.dram_tensor("mbias", (nkv,), f32, kind="ExternalInput").ap()
    if with_bias:
        bq_d = nc.dram_tensor("bq", (1, DL), b16, kind="ExternalInput").ap()
        bk_d = nc.dram_tensor("bk", (1, DL), b16, kind="ExternalInput").ap()
        bv_d = nc.dram_tensor("bv", (1, DL), b16, kind="ExternalInput").ap()
    out_d = nc.dram_tensor("out", (S, D), f32, kind="ExternalOutput").ap()

    with TileContext(nc) as tc:
        with (
            tc.tile_pool(name="persist", bufs=1) as pp,
            tc.tile_pool(name="psum", bufs=2, space="PSUM") as ps2,
            tc.tile_pool(name="etile", bufs=8) as ep,
            tc.tile_pool(name="work", bufs=6) as wp,
            tc.tile_pool(name="ob", bufs=3) as obp,
        ):
            # ---- persistent SBUF tensors ----
            xq_sb = pp.tile([128, NQC, KC8, QCH], b16, tag="xq")
            xkv_sb = pp.tile([128, nkc, KC8, KV_P], b16, tag="xkv")
            wq_sb = pp.tile([128, N_HP, KC8, 128], b16, tag="wq")
            wk_sb = pp.tile([128, N_HP, KC8, 128], b16, tag="wk")
            wv_sb = pp.tile([128, KC8, DL], b16, tag="wv")
            wo_sb = pp.tile([128, DL // 128, D], b16, tag="wo")
            mb_sb = pp.tile([128, nkc], f32, tag="mb")
            qt_sb = pp.tile([128, N_HP, S], b16, tag="qt")
            kt_sb = pp.tile([128, N_HP, nkv], b16, tag="kt")
            # v_aug: [key_part, key_chunk, head, 64 v | 64 ones]
            v_sb = pp.tile([128, nkc, HL, 128], b16, tag="v")
            ctx_sb = pp.tile([128, N_HP, S], b16, tag="ctx")
            if with_bias:
                xq1_sb = pp.tile([1, S], b16, tag="xq1")
                xkv1_sb = pp.tile([1, nkv], b16, tag="xkv1")
                wq1_sb = pp.tile([1, DL], b16, tag="wq1")
                wk1_sb = pp.tile([1, DL], b16, tag="wk1")
                wv1_sb = pp.tile([1, DL], b16, tag="wv1")

            # ---- DMA loads, priority order: everything the first few
            # compute groups need goes first; the rest streams behind ----
            # Few, fat dma_starts: the SP sequencer issues each dma_start
            # serially (~0.6us apiece), so the issue count gates how early
            # the later tensors even begin transferring. First-needed
            # tensors are split in halves so the first projection matmuls
            # can start on the first piece.
            nc.sync.dma_start(mb_sb[:], mb_d.rearrange("(kc p) -> p kc", p=128))
            for h0, h1 in ((0, 4), (4, 8)):
                nc.sync.dma_start(
                    wq_sb[:, 0, h0:h1],
                    wq_d[:, h0 * 128 : h1 * 128].rearrange("p (kc c) -> p kc c", c=128),
                )
                nc.sync.dma_start(
                    xq_sb[:, 0, h0:h1],
                    xq_d[:, h0 * QCH : h1 * QCH].rearrange("p (kc c) -> p kc c", c=QCH),
                )
            nc.sync.dma_start(
                wk_sb[:, 0], wk_d[:, 0 : KC8 * 128].rearrange("p (kc c) -> p kc c", c=128)
            )
            nc.sync.dma_start(
                xkv_sb[:, 0:4],
                xkv_d[:, 0 : 4 * KC8 * KV_P].rearrange(
                    "p (mt kc c) -> p mt kc c", kc=KC8, c=KV_P
                ),
            )
            nc.sync.dma_start(
                wv_sb[:], wv_d.rearrange("p (kc c) -> p kc c", c=DL)
            )
            if with_bias:
                nc.sync.dma_start(wq1_sb[:], bq_d)
                nc.sync.dma_start(wk1_sb[:], bk_d)
                nc.sync.dma_start(wv1_sb[:], bv_d)
            nc.sync.dma_start(
                xkv_sb[:, 4:nkc],
                xkv_d[:, 4 * KC8 * KV_P :].rearrange(
                    "p (mt kc c) -> p mt kc c", kc=KC8, c=KV_P
                ),
            )
            nc.sync.dma_start(
                wk_sb[:, 1:N_HP],
                wk_d[:, KC8 * 128 :].rearrange("p (hp kc c) -> p hp kc c", kc=KC8, c=128),
            )
            nc.sync.dma_start(
                wq_sb[:, 1:N_HP],
                wq_d[:, KC8 * 128 :].rearrange("p (hp kc c) -> p hp kc c", kc=KC8, c=128),
            )
            nc.sync.dma_start(
                xq_sb[:, 1:NQC],
                xq_d[:, KC8 * QCH :].rearrange(
                    "p (qc kc c) -> p qc kc c", kc=KC8, c=QCH
                ),
            )
            nc.sync.dma_start(
                wo_sb[:],
                wo_d.rearrange("p (k c) -> p k c", c=D),
            )

            # constants: ones blocks of v_aug (Pool engine; per-chunk so the
            # first ctx isn't gated on one huge memset). Even heads are
            # [v | ones] (ctx rows 0:63, den 64:127); odd heads are flipped
            # [ones | v] so every DVE op in the normalization reads
            # partition-aligned operands.
            for mt in range(nkc):
                nc.gpsimd.memset(v_sb[:, mt, 0:HL:2, 64:128], 1.0)
                nc.gpsimd.memset(v_sb[:, mt, 1:HL:2, 0:64], 1.0)
            if with_bias:
                nc.vector.memset(xq1_sb[:], 1.0)
                nc.vector.memset(xkv1_sb[:], 1.0)

            # warm the ACT exp table during the DMA lead-in (first use of an
            # activation function costs a ~1.3us table load)
            warm = wp.tile([128, 1], b16, tag="warm", name="warm")
            nc.scalar.activation(warm[:], mb_sb[:, 0:1], EXP)



            # ---- group emitters (each: ~8 matmuls + one copy) ----
            # PSUM budget (8 banks): tag "s" 2 bufs x 2 banks shared by the
            # short-lived tiles (scores, proj, outproj halves); tag "c"
            # 2 bufs x 2 banks for the long-lived ctx accumulators.
            def qproj(hp, qc):
                """qt[:, hp, qc*512:+512] = wq_hp.T @ xq_qc."""
                ps = ps2.tile([128, 512], f32, tag="s", name="ps_q")
                for kc in range(KC8):
                    nc.tensor.matmul(
                        ps[:],
                        lhsT=wq_sb[:, hp, kc, :],
                        rhs=xq_sb[:, qc, kc, :],
                        start=(kc == 0),
                        stop=(not with_bias and kc == KC8 - 1),
                    )
                if with_bias:
                    nc.tensor.matmul(
                        ps[:],
                        lhsT=wq1_sb[:, hp * 128 : hp * 128 + 128],
                        rhs=xq1_sb[:, qc * QCH : qc * QCH + QCH],
                        start=False,
                        stop=True,
                    )
                nc.vector.tensor_copy(
                    out=qt_sb[:, hp, qc * QCH : qc * QCH + QCH], in_=ps[:]
                )

            def kproj(hp, kb):
                """kt[:, hp, off:off+n] for key block kb."""
                off, n = kblocks[kb]
                m0 = off // KV_P
                nmt = n // KV_P
                ps = ps2.tile([128, 512], f32, tag="s", name="ps_k")
                for kc in range(KC8):
                    nc.tensor.matmul(
                        ps[:, :n],
                        lhsT=wk_sb[:, hp, kc, :],
                        rhs=xkv_sb[:, m0 : m0 + nmt, kc, :],
                        start=(kc == 0),
                        stop=(not with_bias and kc == KC8 - 1),
                    )
                if with_bias:
                    nc.tensor.matmul(
                        ps[:, :n],
                        lhsT=wk1_sb[:, hp * 128 : hp * 128 + 128],
                        rhs=xkv1_sb[:, off : off + n],
                        start=False,
                        stop=True,
                    )
                nc.vector.tensor_copy(out=kt_sb[:, hp, off : off + n], in_=ps[:, :n])

            def vproj(mt):
                """v[keys mt*128:+128, all 8 heads] into v_sb."""
                ps = ps2.tile([128, 512], f32, tag="s", name="ps_v")
                for kc in range(KC8):
                    nc.tensor.matmul(
                        ps[:],
                        lhsT=xkv_sb[:, mt, kc, :],
                        rhs=wv_sb[:, kc, :],
                        start=(kc == 0),
                        stop=(not with_bias and kc == KC8 - 1),
                    )
                if with_bias:
                    nc.tensor.matmul(
                        ps[:],
                        lhsT=xkv1_sb[:, mt * KV_P : mt * KV_P + KV_P],
                        rhs=wv1_sb[:],
                        start=False,
                        stop=True,
                    )
                psv = ps[:].rearrange("p (h e) -> p h e", h=HL)
                nc.vector.tensor_copy(out=v_sb[:, mt, 0:HL:2, 0:64], in_=psv[:, 0:HL:2])
                nc.vector.tensor_copy(out=v_sb[:, mt, 1:HL:2, 64:128], in_=psv[:, 1:HL:2])

            def outproj(rt):
                """Full out-projection for row-tile rt (128 q rows): matmuls
                on PE into a "c"-tag tile (shares the rotation with the ctx
                accumulators without perturbing the scores slots), one copy
                on DVE, store via sync DMA."""
                rs = slice(rt * 128, rt * 128 + 128)
                ps = ps2.tile([128, D], f32, tag="c", name="ps_o")
                for nj in range(D // 512):
                    ns = slice(nj * 512, nj * 512 + 512)
                    for khp in range(N_HP):
                        nc.tensor.matmul(
                            ps[:, ns],
                            lhsT=ctx_sb[:, khp, rs],
                            rhs=wo_sb[:, khp, ns],
                            start=(khp == 0),
                            stop=(khp == N_HP - 1),
                        )
                ob = obp.tile([128, D], f32, tag="ob", name="ob")
                nc.vector.tensor_copy(out=ob[:], in_=ps[:])
                nc.sync.dma_start(out_d[rs, :], ob[:])

            # ---- main attention pipeline ----
            steps = [(qc, hp, kc) for qc in range(NQC) for hp in range(N_HP) for kc in range(nkc)]
            nsteps = len(steps)

            # filler schedule: {step_index: [callables]} -- each injected
            # between attention steps, at most ~1 group per step.
            fill = {t: [] for t in range(nsteps + 1)}

            def at(qc, hp, kc):
                return (qc * N_HP + hp) * nkc + min(kc, nkc - 1)

            # Placement rules (consumer must come later in PE program order):
            #   kproj(hp, kb) before scores(qc0, hp, kc=4*kb)
            #   qproj(hp, qc) before scores(qc, hp, kc=0)
            #   vproj(mt)     before ctx(0, 0, mt) at step mt+CTX_LAG
            #   outproj(qc,·) after norm(qc, hp3), i.e. in the (qc+1, ·) iters
            # Pre-phase covers qproj(0,0), kproj(0,0), vproj(0..1).
            fill[at(0, 0, 0)] += [lambda: vproj(0), lambda: vproj(1)]
            for kb in range(1, nkb):  # rest of hp0's keys
                fill[at(0, 0, max(0, 4 * kb - 3))] += [lambda k=kb: kproj(0, k)]
            for mt in range(2, nkc):  # v for all key chunks
                fill[at(0, 0, mt)] += [lambda m=mt: vproj(m)]
            fill[at(0, 0, 5)] += [lambda: qproj(1, 0)]
            fill[at(0, 0, 7)] += [lambda: kproj(1, 0)]
            for hp in range(1, N_HP):
                for kb in range(1, nkb):
                    fill[at(0, hp, max(0, 4 * kb - 3))] += [
                        lambda h=hp, k=kb: kproj(h, k)
                    ]
                if hp + 1 < N_HP:
                    fill[at(0, hp, 4)] += [lambda h=hp + 1: qproj(h, 0)]
                    fill[at(0, hp, 6)] += [lambda h=hp + 1: kproj(h, 0)]
            # remaining qproj groups: one per iteration, one iteration ahead
            # of consumption
            for qc in range(1, NQC):
                for hp in range(N_HP):
                    lin = qc * N_HP + hp - 1
                    pq, ph = divmod(lin, N_HP)
                    fill[at(pq, ph, 3 if pq > 0 else 5)] += [
                        lambda h=hp, q=qc: qproj(h, q)
                    ]
            # out-projection row-tiles spread over the next qc's iterations;
            # kept >= one iteration past norm(qc, hp3) so the PE never waits
            # on the DVE normalization chain
            for qc in range(NQC - 1):
                rts = [qc * 4 + i for i in range(4)]
                fill[at(qc + 1, 1, 1)] += [lambda r=rts[0]: outproj(r)]
                fill[at(qc + 1, 1, 6)] += [lambda r=rts[1]: outproj(r)]
                fill[at(qc + 1, 2, 1)] += [lambda r=rts[2]: outproj(r)]
                fill[at(qc + 1, 2, 6)] += [lambda r=rts[3]: outproj(r)]

            # pre-phase: minimum work to start the first scores/exp. The
            # junk matmuls between the groups keep the Tensor engine busy
            # through the DMA waits (it runs at ~half clock until it has
            # executed continuously for ~3us, and idle gaps reset the ramp).
            jnk = wp.tile([128, 512], b16, tag="jnk", name="jnk")
            nc.vector.memset(jnk[:], 0.0)

            def junk_mms(n):
                jps = ps2.tile([128, 512], f32, tag="s", name="jps")
                for i in range(n):
                    nc.tensor.matmul(
                        jps[:],
                        lhsT=jnk[:, 0:128],
                        rhs=jnk[:],
                        start=(i == 0),
                        stop=(i == n - 1),
                    )

            qproj(0, 0)
            junk_mms(8)
            kproj(0, 0)
            junk_mms(6)

            cc_tiles = {}  # (qc, hp) -> psum tile
            pending = []  # (e01, qc, hp, kc)

            def ctx_mm(e01, qc, hp, kc):
                key = (qc, hp)
                if key not in cc_tiles:
                    cc_tiles[key] = ps2.tile([128, 2 * QCH], f32, tag="c", name="cc")
                cc = cc_tiles[key]
                nc.tensor.matmul(
                    cc[:, 0:QCH],
                    lhsT=v_sb[:, kc, 2 * hp, :],
                    rhs=e01[:, 0:QCH],
                    start=(kc == 0),
                    stop=(kc == nkc - 1),
                )
                nc.tensor.matmul(
                    cc[:, QCH : 2 * QCH],
                    lhsT=v_sb[:, kc, 2 * hp + 1, :],
                    rhs=e01[:, QCH : 2 * QCH],
                    start=(kc == 0),
                    stop=(kc == nkc - 1),
                )
                if kc == nkc - 1:
                    norm(qc, hp)
                    del cc_tiles[key]

            def norm(qc, hp):
                """ctx_sb[:, hp, qs] = ctx / den.
                Head-even (c0): ctx rows 0:63, den rows 64:127; head-odd (c1)
                flipped: den 0:63, ctx 64:127. Both dens relocate into one
                [128, 512] SBUF tile -> a single full-width
                reciprocal_approx_fast (base-partition-0, unshifted) covers
                both heads, and both multiplies read partition-aligned
                operands."""
                qs = slice(qc * QCH, qc * QCH + QCH)
                cc = cc_tiles[(qc, hp)]
                c0 = cc[:, 0:QCH]
                c1 = cc[:, QCH : 2 * QCH]
                den = wp.tile([128, QCH], f32, tag="den", name="den")
                nc.vector.tensor_copy(out=den[0:64, :], in_=c0[64:128, :])
                nc.vector.tensor_copy(out=den[64:128, :], in_=c1[0:64, :])
                rc = wp.tile([128, QCH], f32, tag="rc", name="rc")
                nc.vector.reciprocal_approx_fast(rc[:], den[:])
                nc.vector.tensor_mul(
                    out=ctx_sb[0:64, hp, qs], in0=c0[0:64, :], in1=rc[0:64, :]
                )
                nc.vector.tensor_mul(
                    out=ctx_sb[64:128, hp, qs], in0=c1[64:128, :], in1=rc[64:128, :]
                )

            def scores_exp(t):
                qc, hp, kc = steps[t]
                qs = slice(qc * QCH, qc * QCH + QCH)
                ks = slice(kc * KV_P, kc * KV_P + KV_P)
                s01 = ps2.tile([128, 2 * QCH], f32, tag="s", name="s01")
                nc.tensor.matmul(
                    s01[:, 0:QCH],
                    lhsT=kt_sb[0:64, hp, ks],
                    rhs=qt_sb[0:64, hp, qs],
                )
                nc.tensor.matmul(
                    s01[:, QCH : 2 * QCH],
                    lhsT=kt_sb[64:128, hp, ks],
                    rhs=qt_sb[64:128, hp, qs],
                )
                e01 = ep.tile([128, 2 * QCH], b16, tag="e", name="e01")
                nc.scalar.activation(
                    e01[:],
                    s01[:],
                    EXP,
                    bias=mb_sb[:, kc : kc + 1],
                    scale=SCALE,
                )
                pending.append((e01, qc, hp, kc))

            # steps run in PAIRS: both scores+exp first (filling both PSUM
            # scores slots, so ACT has ~2us of queued exp work), then the
            # ctx pops, then fills. This keeps ACT fed across a fill group
            # and keeps the PE from waiting on exp at the next pair.
            # Pops stay before fills: a fill may read state (ctx_sb via
            # norm) that the popped ctx/norm writes.
            for p in range(0, nsteps, 2):
                pair = [t for t in (p, p + 1) if t < nsteps]
                for t in pair:
                    scores_exp(t)
                while len(pending) > CTX_LAG:
                    ctx_mm(*pending.pop(0))
                for t in pair:
                    for f in fill.get(t, ()):
                        f()

            for p in pending:
                ctx_mm(*p)

            # tail: out-projection for the last q-chunk's rows
            for rt in range(4 * (NQC - 1), 4 * NQC):
                outproj(rt)

    nc.finalize()
    return nc


def _host_prep(x, mask, wq, bq, wk, bk, wv, bv, wo):
    x = np.asarray(x, dtype=np.float32)
    mask = np.asarray(mask)
    # per-batch gather of unmasked keys
    idxs = [np.nonzero(mask[b])[0] for b in range(B)]
    nmax = max(1, max(len(i) for i in idxs))
    nkv = min(S, ((nmax + KV_P - 1) // KV_P) * KV_P)
    nkc = nkv // KV_P
    with_bias = bool(
        np.any(np.asarray(bq)) or np.any(np.asarray(bk)) or np.any(np.asarray(bv))
    )

    def pack_x(xt, inner):
        """[D, n] -> [128, (n//inner) * KC8 * inner]: outer-major blocks of
        `inner` columns, kc-major within a block, contiguous per partition."""
        d, n = xt.shape
        nb = n // inner
        a = xt.reshape(KC8, 128, nb, inner).transpose(1, 2, 0, 3)
        return np.ascontiguousarray(a).reshape(128, nb * KC8 * inner).astype(bf16)

    def pack_w(wt):
        """[D, 512] -> [128, N_HP * KC8 * 128] (hp-major, kc, 128)."""
        a = wt.reshape(KC8, 128, N_HP, 128).transpose(1, 2, 0, 3)
        return np.ascontiguousarray(a).reshape(128, N_HP * KC8 * 128).astype(bf16)

    def pack_kc(a, kcount):
        """[kcount*128, n] -> [128, kcount*n] kc-major per partition."""
        k128, n = a.shape
        return (
            np.ascontiguousarray(a.reshape(kcount, 128, n).transpose(1, 0, 2))
            .reshape(128, kcount * n)
            .astype(bf16)
        )

    in_maps = []
    for c in range(DP * TP):
        b, g = c // TP, c % TP
        sl = slice(g * DL, g * DL + DL)

        idx = idxs[b]
        xg = np.zeros((nkv, D), dtype=np.float32)
        xg[: len(idx)] = x[b][idx]

        mbias = np.full((nkv,), NEG, dtype=np.float32)
        mbias[: len(idx)] = 0.0

        im = {
            "xq": pack_x(x[b].T, QCH),
            "xkv": pack_x(xg.T, KV_P),
            "wqt": pack_w(np.asarray(wq)[sl, :].T),
            "wkt": pack_w(np.asarray(wk)[sl, :].T),
            "wvt": pack_kc(np.asarray(wv)[sl, :].T, KC8),
            "wot": pack_kc(np.asarray(wo)[:, sl].T, DL // 128),
            "mbias": mbias,
        }
        if with_bias:
            im["bq"] = np.asarray(bq)[None, sl].astype(bf16)
            im["bk"] = np.asarray(bk)[None, sl].astype(bf16)
            im["bv"] = np.asarray(bv)[None, sl].astype(bf16)
        in_maps.append(im)
    return nkv, with_bias, in_maps


def kernel(x, mask, wq, bq, wk, bk, wv, bv, wo, bo):
    from concourse.bass_utils import run_bass_kernel_spmd

    nkv, with_bias, in_maps = _host_prep(x, mask, wq, bq, wk, bk, wv, bv, wo)
    nc = _build(nkv, with_bias)
    res = run_bass_kernel_spmd(nc, in_maps, core_ids=list(range(DP * TP)))

    out = np.empty((B, S, D), dtype=np.float32)
    bo = np.asarray(bo, dtype=np.float32)
    for b in range(B):
        out[b] = res.results[b * TP]["out"] + res.results[b * TP + 1]["out"] + bo
    return out
